# revision 1
# baseline (speedup 1.0000x reference)
"""Contrastive volume loss (nn_ContrastiveVolumeLoss) on 8 Trainium2 cores.

Reference math:
  ind_k = floor(locations_k) @ [W, 1]
  G     = [emb_0.reshape(c,HW)[:, ind_0] | emb_1.reshape(c,HW)[:, ind_1]]
  sim   = G^T G                       (2n x 2n, G is channel-major (64, 8192))
  S_i   = sum_j exp(sim_ij / T) - e^(1/T)
  loss  = (sum_i log S_i - (2/T) sum_u sim[u, u+n]) / (2n)

Sharding (variant of the row-block hint): the host computes the indices and
slices the gathered point embeddings out of the inputs (pure data staging --
index-select plus bf16 cast of 2 MiB; all O(n^2) compute and traffic stays on
device). The 8192x8192 sim matrix is symmetric, so each core computes an
upper-trapezoid slice: it owns 8 row-tiles of 128 rows, one from each
diagonal work class (slot k's tile needs column regions j >= JD[k], regions
are 1024 wide), giving all cores identical instruction streams on different
rows -- required for SPMD -- and a balanced 36 of the 64x8 region chunks.

Per chunk (128 rows x 1024 cols): 2 bf16 matmuls (K=64) into PSUM, then one
fused scalar-engine pass computes exp(10*sim) with the row-sum emitted via
the activation accumulator (scale=1/T folds the temperature in; the exp
values themselves are only kept transiently in bf16 scratch). The lower
triangle is recovered from column sums of the strictly-above-diagonal
chunks: ones-vector matmuls accumulated per region in PSUM. The
positive-pair term reduces to dot products lhs.par, computed with an
elementwise multiply and a ones-matmul. The host adds log / final reduction
over 8192 scalars and the symmetry bookkeeping.
"""

import numpy as np
import ml_dtypes

import concourse.bacc as bacc
import concourse.mybir as mybir
from concourse.tile import TileContext
from concourse.bass_utils import run_bass_kernel_spmd

N_CORES = 8
C = 64
HW = 256 * 256
N_PTS = 4096
TWO_N = 2 * N_PTS
T_INV = 10.0
W_IMG = 256

JD = [0, 7, 1, 6, 2, 5, 3, 4]      # diagonal region per slot
N_SLOTS = 8
REGION = 1024                       # column region width
N_REGIONS = TWO_N // REGION         # 8

_BF16 = ml_dtypes.bfloat16
_PROGRAM_CACHE = {}


def _slot_tiles(r):
    """Global 128-row tile indices owned by core r, in slot order."""
    return [r, 63 - r, 8 + r, 55 - r, 16 + r, 47 - r, 24 + r, 39 - r]


def _build_program():
    nc = bacc.Bacc(
        "TRN2", target_bir_lowering=False, debug=False, num_devices=N_CORES
    )
    lhs_d = nc.dram_tensor("lhs", [C, 1024], mybir.dt.bfloat16,
                           kind="ExternalInput")
    rhs_d = nc.dram_tensor("rhs", [C, TWO_N], mybir.dt.bfloat16,
                           kind="ExternalInput")
    par_d = nc.dram_tensor("par", [C, 1024], mybir.dt.bfloat16,
                           kind="ExternalInput")
    ones_d = nc.dram_tensor("ones", [128, 1], mybir.dt.bfloat16,
                            kind="ExternalInput")
    rs_d = nc.dram_tensor("rowsums", [128, N_SLOTS], mybir.dt.float32,
                          kind="ExternalOutput")
    cs_d = nc.dram_tensor("colsums", [1, TWO_N - REGION], mybir.dt.float32,
                          kind="ExternalOutput")
    pos_d = nc.dram_tensor("pos", [1, 2], mybir.dt.float32,
                           kind="ExternalOutput")

    with TileContext(nc) as tc:
        with (
            tc.tile_pool(name="const", bufs=1) as cpool,
            tc.tile_pool(name="work", bufs=4) as wpool,
            tc.tile_pool(name="psum", bufs=3, space="PSUM") as ppool,
            tc.tile_pool(name="cs", bufs=2, space="PSUM") as cspool,
        ):
            # Regions processed in descending chunk-count order: the busiest
            # region fills the pipe first and the lightest lands in the tail.
            j_order = [1, 7, 6, 5, 4, 3, 2, 0]

            # lhs/ones/par go over the scalar engine's HWDGE ring so they
            # don't delay the rhs region loads on the sync ring.
            lhs_t = cpool.tile([C, 1024], mybir.dt.bfloat16, tag="lhs")
            nc.scalar.dma_start(lhs_t[:], lhs_d[:])
            ones_t = cpool.tile([128, 1], mybir.dt.bfloat16, tag="ones")
            nc.scalar.dma_start(ones_t[:], ones_d[:])
            par_t = cpool.tile([C, 1024], mybir.dt.bfloat16, tag="par")
            nc.scalar.dma_start(par_t[:], par_d[:])
            rhs_ts = {}
            for j in j_order:
                rt = cpool.tile([C, REGION], mybir.dt.bfloat16, tag=f"rhs{j}")
                nc.sync.dma_start(rt[:], rhs_d[:, j * REGION:(j + 1) * REGION])
                rhs_ts[j] = rt

            # Dummy 1-element exp so the ~2us exp table load happens during
            # the input-DMA window instead of stalling the first real chunk.
            warm_t = cpool.tile([1, 1], mybir.dt.float32, tag="warm")
            nc.gpsimd.memset(warm_t[:], 0.0)
            nc.scalar.activation(warm_t[:], warm_t[:],
                                 mybir.ActivationFunctionType.Exp, scale=1.0)

            # Positive-pair dots: pos[u] = lhs[:, u] . par[:, u].
            prod_t = cpool.tile([C, 1024], mybir.dt.bfloat16, tag="prod")
            nc.vector.tensor_mul(prod_t[:], lhs_t[:], par_t[:])
            pos_sb = cpool.tile([1, 2], mybir.dt.float32, tag="pos_sb")
            for h in range(2):
                pp = cspool.tile([1, 512], mybir.dt.float32, tag="cs")
                nc.tensor.matmul(pp[:], ones_t[:C, :],
                                 prod_t[:, h * 512:(h + 1) * 512],
                                 start=True, stop=True)
                nc.vector.tensor_reduce(pos_sb[:, h:h + 1], pp[:],
                                        axis=mybir.AxisListType.X,
                                        op=mybir.AluOpType.add)
            nc.sync.dma_start(pos_d[:], pos_sb[:])

            # Row-sum accumulator: column (k*8 + j) holds the fused exp sum
            # of chunk (slot k, region j).
            acc = cpool.tile([128, N_SLOTS * N_REGIONS], mybir.dt.float32,
                             tag="acc")
            cs_sb = cpool.tile([1, TWO_N - REGION], mybir.dt.float32,
                               tag="cs_sb")

            for j in j_order:
                strict = [k for k in range(N_SLOTS) if JD[k] < j]
                cs_ps = None
                if strict:
                    cs_ps0 = cspool.tile([1, 512], mybir.dt.float32, tag="cs")
                    cs_ps1 = cspool.tile([1, 512], mybir.dt.float32, tag="cs")
                    cs_ps = [cs_ps0, cs_ps1]
                for ki, k in enumerate(k for k in range(N_SLOTS) if JD[k] <= j):
                    ps = ppool.tile([128, REGION], mybir.dt.float32, tag="ps")
                    lhsT = lhs_t[:, k * 128:(k + 1) * 128]
                    for h in range(2):
                        nc.tensor.matmul(
                            ps[:, h * 512:(h + 1) * 512], lhsT,
                            rhs_ts[j][:, h * 512:(h + 1) * 512],
                            start=True, stop=True)
                    scratch = wpool.tile([128, REGION], mybir.dt.bfloat16,
                                         tag="scratch")
                    nc.scalar.activation(
                        scratch[:], ps[:], mybir.ActivationFunctionType.Exp,
                        scale=T_INV, accum_out=acc[:, k * 8 + j:k * 8 + j + 1])
                    if JD[k] < j:
                        si = [x for x in strict].index(k)
                        for h in range(2):
                            nc.tensor.matmul(
                                cs_ps[h][:], ones_t[:],
                                scratch[:, h * 512:(h + 1) * 512],
                                start=(si == 0), stop=(si == len(strict) - 1))
                if strict:
                    base = (j - 1) * REGION
                    for h in range(2):
                        nc.vector.tensor_copy(
                            cs_sb[:, base + h * 512:base + (h + 1) * 512],
                            cs_ps[h][:])

            rs_sb = cpool.tile([128, N_SLOTS], mybir.dt.float32, tag="rs_sb")
            for k in range(N_SLOTS):
                nc.vector.tensor_reduce(
                    rs_sb[:, k:k + 1], acc[:, k * 8 + JD[k]:k * 8 + 8],
                    axis=mybir.AxisListType.X, op=mybir.AluOpType.add)
            nc.sync.dma_start(rs_d[:], rs_sb[:])
            nc.sync.dma_start(cs_d[:], cs_sb[:])

    nc.compile()
    return nc


def kernel(emb_0, emb_1, locations_0, locations_1):
    emb_0 = np.asarray(emb_0)
    emb_1 = np.asarray(emb_1)
    locations_0 = np.asarray(locations_0)
    locations_1 = np.asarray(locations_1)

    strides = np.array([W_IMG, 1], dtype=np.float32)
    ind0 = (np.floor(locations_0[0]) @ strides).astype(np.int32)
    ind1 = (np.floor(locations_1[0]) @ strides).astype(np.int32)

    g0 = emb_0.reshape(C, HW)[:, ind0]
    g1 = emb_1.reshape(C, HW)[:, ind1]
    G = np.concatenate([g0, g1], axis=1).astype(_BF16)   # (64, 8192)
    P = np.concatenate([g1, g0], axis=1).astype(_BF16)   # partner columns

    if "nc" not in _PROGRAM_CACHE:
        _PROGRAM_CACHE["nc"] = _build_program()
    nc = _PROGRAM_CACHE["nc"]

    ones = np.ones((128, 1), dtype=_BF16)
    in_maps = []
    row_of = np.empty((N_CORES, 1024), dtype=np.int64)
    for r in range(N_CORES):
        tiles = _slot_tiles(r)
        rows = np.concatenate(
            [np.arange(mt * 128, (mt + 1) * 128) for mt in tiles])
        row_of[r] = rows
        in_maps.append({
            "lhs": np.ascontiguousarray(G[:, rows]),
            "rhs": G,
            "par": np.ascontiguousarray(P[:, rows]),
            "ones": ones,
        })

    res = run_bass_kernel_spmd(nc, in_maps, core_ids=list(range(N_CORES)))

    rowsum = np.zeros(TWO_N, dtype=np.float64)
    pos_total = 0.0
    for r in range(N_CORES):
        rs = res.results[r]["rowsums"].astype(np.float64)   # (128, 8 slots)
        rowsum[row_of[r]] += rs.T.reshape(-1)               # slot-major rows
        rowsum[REGION:] += res.results[r]["colsums"][0].astype(np.float64)
        pos_total += float(np.sum(res.results[r]["pos"]))

    sums = rowsum - float(np.exp(np.float32(T_INV), dtype=np.float32))
    loss = (np.sum(np.log(sums)) - T_INV * pos_total) / TWO_N
    return np.float32(loss)



# revision 2
# speedup vs baseline: 1.1292x; 1.1292x over previous
"""Contrastive volume loss (nn_ContrastiveVolumeLoss) on 8 Trainium2 cores.

Reference math:
  ind_k = floor(locations_k) @ [W, 1]
  G     = [emb_0.reshape(c,HW)[:, ind_0] | emb_1.reshape(c,HW)[:, ind_1]]
  sim   = G^T G                       (2n x 2n, G is channel-major (64, 8192))
  S_i   = sum_j exp(sim_ij / T) - e^(1/T)
  loss  = (sum_i log S_i - (2/T) sum_u sim[u, u+n]) / (2n)

Sharding: the host computes indices and slices the gathered point embeddings
(pure data staging). Each core owns 8 row-tiles of 128 rows (one per diagonal
work class, slot k's tile has diag region JD[k]); it computes the upper
trapezoid of the symmetric 8192x8192 sim matrix: slot k covers columns
[JD[k]*1024, 8192). Row sums of exp come from the activation accumulator or
DVE reduces; the lower triangle is recovered from column sums computed with
near-free tiny matmuls (exp-scratch as PE weights x ones, N=1) accumulated
in a pinned PSUM bank; the host merges rowsums + colsums + positive pairs.

Engine split (the scalar/Act engine is the PSUM-exp bottleneck; DVE is the
only other engine that can read PSUM):
  - Act: exact exp with fused row-sum accumulation on ~56% of columns.
  - DVE: Schraudolph approximate exp on the rest - one tensor_scalar per
    1024-col window computes int16 bits = sim*(T_inv*128/ln2) + 128*(127+s),
    which bitcast as bf16 is exp(sim/T) to ~1.8% rms (bias tuned via SIGMA).
  - Pool (gpsimd): bf16 add-trees combine 4 Schraudolph windows so DVE pays
    only one row-sum reduce per 4096 columns.
  - PE: bf16 sim matmuls + free colsum/pos tiny matmuls (cost model charges
    matmuls by output free size only).
"""

import numpy as np
import ml_dtypes

import concourse.bacc as bacc
import concourse.mybir as mybir
from concourse.tile import TileContext
from concourse.bass_utils import run_bass_kernel_spmd

N_CORES = 8
C = 64
HW = 256 * 256
N_PTS = 4096
TWO_N = 2 * N_PTS
T_INV = 10.0
W_IMG = 256

JD = [0, 7, 1, 6, 2, 5, 3, 4]      # diagonal region per slot
N_SLOTS = 8

SIGMA = -0.0579
A_CONST = float(T_INV * 128.0 / np.log(2.0))
B_CONST = float(128.0 * (127.0 + SIGMA))

_BF16 = ml_dtypes.bfloat16
_PROGRAM_CACHE = {}

# Per-slot window plan. 'A' windows -> Act exact exp (+accum rowsum);
# 'G4' -> four 1024-col DVE Schraudolph windows + Pool add tree + DVE reduce.
# Sizes are columns relative to the slot start (global col JD[k]*1024).
PLAN = {
    0: [("G4", 4096), ("A", 1536), ("A", 1536), ("A", 1024)],
    1: [("A", 1024)],
    2: [("G4", 4096), ("A", 1536), ("A", 1536)],
    3: [("A", 1024), ("A", 1024)],
    4: [("G4", 4096), ("A", 1024), ("A", 1024)],
    5: [("A", 1536), ("A", 1536)],
    6: [("A", 1536), ("A", 1536), ("A", 1024), ("A", 1024)],
    7: [("G4", 4096)],
}

N_CS = 56        # colsum accumulator columns (global block g -> col g-8)
POS_OFF = N_CS   # 8 pos columns after the colsums


def _slot_tiles(r):
    """Global 128-row tile indices owned by core r, in slot order."""
    return [r, 63 - r, 8 + r, 55 - r, 16 + r, 47 - r, 24 + r, 39 - r]


def _windows():
    """Flat window list: (slot, glo, width, kind, acc_col, group_id)."""
    out = []
    for k in range(N_SLOTS):
        base = JD[k] * 1024
        off = 0
        acc_i = 0
        for kind, w in PLAN[k]:
            if kind == "A":
                out.append((k, base + off, w, "A", k * 4 + acc_i, None))
                acc_i += 1
                off += w
            else:  # G4: four 1024 windows sharing one acc col
                gid = (k, acc_i)
                for j in range(4):
                    out.append((k, base + off, 1024, "D", k * 4 + acc_i, gid))
                    off += 1024
                acc_i += 1
        assert JD[k] * 1024 + off == TWO_N if off else True
    return out


def _emission_order(wins):
    """Interleave pA (Act >=1280) windows with pB/pC windows so the three
    PSUM pools pipeline: pattern [D, A_big, D, A_small] x 8."""
    d = [w for w in wins if w[3] == "D"]
    a_big = [w for w in wins if w[3] == "A" and w[2] > 1024]
    a_small = [w for w in wins if w[3] == "A" and w[2] <= 1024]
    order = []
    di = bi = si = 0
    while di < len(d) or bi < len(a_big) or si < len(a_small):
        if di < len(d):
            order.append(d[di]); di += 1
        if bi < len(a_big):
            order.append(a_big[bi]); bi += 1
        if di < len(d):
            order.append(d[di]); di += 1
        if si < len(a_small):
            order.append(a_small[si]); si += 1
    return order


def _cs_contributors(order):
    """Map colsum column g -> ordered list of window indices contributing."""
    contrib = {}
    for wi, (k, glo, w, kind, _, _) in enumerate(order):
        for b in range(w // 128):
            g = (glo // 128) + b
            if g // 8 == JD[k]:
                continue  # diagonal region: rowsum-only
            contrib.setdefault(g, []).append(wi)
    return contrib


def _build_program():
    nc = bacc.Bacc(
        "TRN2", target_bir_lowering=False, debug=False, num_devices=N_CORES
    )
    lhs_d = nc.dram_tensor("lhs", [C, 1024], mybir.dt.bfloat16,
                           kind="ExternalInput")
    rhs_d = nc.dram_tensor("rhs", [C, TWO_N], mybir.dt.bfloat16,
                           kind="ExternalInput")
    par_d = nc.dram_tensor("par", [C, 1024], mybir.dt.bfloat16,
                           kind="ExternalInput")
    ones_d = nc.dram_tensor("ones", [128, 1], mybir.dt.bfloat16,
                            kind="ExternalInput")
    rs_d = nc.dram_tensor("rowsums", [128, 32], mybir.dt.float32,
                          kind="ExternalOutput")
    cs_d = nc.dram_tensor("colsums", [128, 64], mybir.dt.float32,
                          kind="ExternalOutput")

    order = _emission_order(_windows())
    contrib = _cs_contributors(order)
    first_of = {g: ws[0] for g, ws in contrib.items()}
    last_of = {g: ws[-1] for g, ws in contrib.items()}

    with TileContext(nc) as tc:
        with (
            tc.tile_pool(name="const", bufs=1) as cpool,
            tc.tile_pool(name="ascr", bufs=2) as apool,
            tc.tile_pool(name="dscr", bufs=5) as dpool,
            tc.tile_pool(name="addp", bufs=2) as addpool,
            tc.tile_pool(name="pin", bufs=1, space="PSUM") as pinpool,
            tc.tile_pool(name="pa", bufs=1, space="PSUM") as pa,
            tc.tile_pool(name="pb", bufs=1, space="PSUM") as pb,
            tc.tile_pool(name="pc", bufs=1, space="PSUM") as pc,
        ):
            # ---- constants / inputs ----
            lhs_t = cpool.tile([C, 1024], mybir.dt.bfloat16, tag="lhs")
            nc.scalar.dma_start(lhs_t[:], lhs_d[:])
            ones_t = cpool.tile([128, 1], mybir.dt.bfloat16, tag="ones")
            nc.scalar.dma_start(ones_t[:], ones_d[:])
            par_t = cpool.tile([C, 1024], mybir.dt.bfloat16, tag="par")
            nc.scalar.dma_start(par_t[:], par_d[:])
            rhs_t = cpool.tile([C, TWO_N], mybir.dt.bfloat16, tag="rhs")
            for j in range(4):
                nc.sync.dma_start(rhs_t[:, j * 2048:(j + 1) * 2048],
                                  rhs_d[:, j * 2048:(j + 1) * 2048])

            # exp act-table warm-up during the input DMA window
            warm_t = cpool.tile([1, 1], mybir.dt.float32, tag="warm")
            nc.gpsimd.memset(warm_t[:], 0.0)
            nc.scalar.activation(warm_t[:], warm_t[:],
                                 mybir.ActivationFunctionType.Exp, scale=1.0)

            # rowsum accumulator columns (<=4 per slot) and pinned colsum bank
            acc = cpool.tile([128, 32], mybir.dt.float32, tag="acc")
            nc.gpsimd.memset(acc[:], 0.0)
            cs_acc = pinpool.tile([128, 64], mybir.dt.float32, tag="cs")

            # ---- positive pairs: prod = lhs .* par, tiny-matmul col dots ----
            prod_t = cpool.tile([C, 1024], mybir.dt.bfloat16, tag="prod")
            nc.vector.tensor_tensor(prod_t[:], lhs_t[:], par_t[:],
                                    mybir.AluOpType.mult)
            for b in range(8):
                nc.tensor.matmul(cs_acc[:, POS_OFF + b:POS_OFF + b + 1],
                                 prod_t[:, b * 128:(b + 1) * 128],
                                 ones_t[:C, :], start=True, stop=True)

            # ---- streamed windows ----
            group_scr = {}   # gid -> list of int16 scratch tiles
            pbc = [pb, pc]
            pbc_i = 0
            for wi, (k, glo, w, kind, acol, gid) in enumerate(order):
                lhsT = lhs_t[:, k * 128:(k + 1) * 128]
                if kind == "A" and w > 1024:
                    ptile = pa.tile([128, 1536], mybir.dt.float32, tag="pa")
                else:
                    pool_ = pbc[pbc_i % 2]
                    pbc_i += 1
                    ptile = pool_.tile([128, 1024], mybir.dt.float32,
                                       tag=f"p{'bc'[pbc_i % 2]}")
                nmm = (w + 511) // 512
                for h in range(nmm):
                    cw = min(512, w - h * 512)
                    nc.tensor.matmul(ptile[:, h * 512:h * 512 + cw], lhsT,
                                     rhs_t[:, glo + h * 512:glo + h * 512 + cw],
                                     start=True, stop=True)

                if kind == "A":
                    scr = apool.tile([128, w], mybir.dt.bfloat16,
                                     tag=f"as{w}")
                    nc.scalar.activation(
                        scr[:], ptile[:, :w],
                        mybir.ActivationFunctionType.Exp, scale=T_INV,
                        accum_out=acc[:, acol:acol + 1])
                    scr_b = scr
                else:
                    scr = dpool.tile([128, 1024], mybir.dt.int16, tag="ds")
                    nc.vector.tensor_scalar(
                        scr[:], ptile[:, :w], A_CONST, B_CONST,
                        mybir.AluOpType.mult, mybir.AluOpType.add)
                    group_scr.setdefault(gid, []).append(scr)
                    scr_b = None

                # colsum tiny matmuls (free on PE): scratch block as weights
                sb = scr[:].bitcast(mybir.dt.bfloat16) if kind == "D" else scr_b[:]
                for b in range(w // 128):
                    g = (glo // 128) + b
                    if g // 8 == JD[k]:
                        continue
                    nc.tensor.matmul(
                        cs_acc[:, g - 8:g - 7],
                        sb[:, b * 128:(b + 1) * 128], ones_t[:],
                        start=(first_of[g] == wi), stop=(last_of[g] == wi))

                # close out a finished G4 group: Pool add tree + DVE reduce
                if gid is not None and len(group_scr.get(gid, ())) == 4:
                    s0, s1, s2, s3 = [t[:].bitcast(mybir.dt.bfloat16)
                                      for t in group_scr.pop(gid)]
                    s01 = addpool.tile([128, 1024], mybir.dt.bfloat16,
                                       tag="s01")
                    s23 = addpool.tile([128, 1024], mybir.dt.bfloat16,
                                       tag="s23")
                    stot = addpool.tile([128, 1024], mybir.dt.bfloat16,
                                        tag="stot")
                    nc.gpsimd.tensor_tensor(s01[:], s0, s1,
                                            mybir.AluOpType.add)
                    nc.gpsimd.tensor_tensor(s23[:], s2, s3,
                                            mybir.AluOpType.add)
                    nc.gpsimd.tensor_tensor(stot[:], s01[:], s23[:],
                                            mybir.AluOpType.add)
                    nc.vector.tensor_reduce(acc[:, acol:acol + 1], stot[:],
                                            axis=mybir.AxisListType.X,
                                            op=mybir.AluOpType.add)

            # ---- outputs ----
            cs_sb = cpool.tile([128, 64], mybir.dt.float32, tag="cs_sb")
            nc.vector.tensor_copy(cs_sb[:], cs_acc[:])
            nc.sync.dma_start(rs_d[:], acc[:])
            nc.sync.dma_start(cs_d[:], cs_sb[:])

    nc.compile()
    return nc


def kernel(emb_0, emb_1, locations_0, locations_1):
    emb_0 = np.asarray(emb_0)
    emb_1 = np.asarray(emb_1)
    locations_0 = np.asarray(locations_0)
    locations_1 = np.asarray(locations_1)

    strides = np.array([W_IMG, 1], dtype=np.float32)
    ind0 = (np.floor(locations_0[0]) @ strides).astype(np.int32)
    ind1 = (np.floor(locations_1[0]) @ strides).astype(np.int32)

    g0 = emb_0.reshape(C, HW)[:, ind0]
    g1 = emb_1.reshape(C, HW)[:, ind1]
    G = np.concatenate([g0, g1], axis=1).astype(_BF16)   # (64, 8192)
    P = np.concatenate([g1, g0], axis=1).astype(_BF16)   # partner columns

    if "nc" not in _PROGRAM_CACHE:
        _PROGRAM_CACHE["nc"] = _build_program()
    nc = _PROGRAM_CACHE["nc"]

    ones = np.ones((128, 1), dtype=_BF16)
    in_maps = []
    row_of = np.empty((N_CORES, 1024), dtype=np.int64)
    for r in range(N_CORES):
        tiles = _slot_tiles(r)
        rows = np.concatenate(
            [np.arange(mt * 128, (mt + 1) * 128) for mt in tiles])
        row_of[r] = rows
        in_maps.append({
            "lhs": np.ascontiguousarray(G[:, rows]),
            "rhs": G,
            "par": np.ascontiguousarray(P[:, rows]),
            "ones": ones,
        })

    res = run_bass_kernel_spmd(nc, in_maps, core_ids=list(range(N_CORES)))

    rowsum = np.zeros(TWO_N, dtype=np.float64)
    pos_total = 0.0
    for r in range(N_CORES):
        rows = row_of[r]
        rs = res.results[r]["rowsums"].astype(np.float64)   # (128, 32)
        for k in range(N_SLOTS):
            srow = rs[:, k * 4:(k + 1) * 4].sum(axis=1)     # (128,)
            rowsum[rows[k * 128:(k + 1) * 128]] += srow
        cs = res.results[r]["colsums"].astype(np.float64)   # (128, 64)
        for col in range(N_CS):
            g = col + 8
            rowsum[g * 128:(g + 1) * 128] += cs[:, col]
        pos_total += float(cs[:, POS_OFF:POS_OFF + 8].sum())

    sums = rowsum - float(np.exp(np.float32(T_INV), dtype=np.float32))
    loss = (np.sum(np.log(sums)) - T_INV * pos_total) / TWO_N
    return np.float32(loss)
